# revision 33
# baseline (speedup 1.0000x reference)
"""Trainium2 Bass kernel for nn_CumulantSOAP_CV: per-column cumulants of
X (100000, 1024) up to order 5, then (X_cum - mu) @ W -> (1, 8).

Strategy (8 NeuronCores, SPMD):
  - Host pads X to 100352 rows (zeros don't perturb power sums) and shards
    rows across 8 cores: 12544 rows = 98 tiles of (128, 1024) each.
  - Per core, one pass over X: DMA-cast f32->bf16 (SWDGE), then per tile
    compute x^2 (ScalarE Square), x^3 = x*x^2 and x^5 = x^2*x^3 (VectorE).
    Column sums of x, x^2, x^3, x^5 via ones-vector matmuls accumulated in
    PSUM, issued at 4 distinct 32-col PE strips so they run concurrently.
    S4 = sum((x^2)^2) via PE "diagonal" matmuls x2_chunk^T @ x2_chunk
    (128-col chunks) accumulated in PSUM; diagonal extracted at the end.
  - 5x1024 raw power sums are AllReduduced across the 8 cores, cumulants
    combined from raw moments, and the (1,5120)@(5120,8) projection done
    replicated on every core. Output taken from core 0.
"""

import numpy as np

import concourse.bass as bass
import concourse.mybir as mybir
import concourse.tile as tile
from concourse import bacc
from concourse.bass_utils import run_bass_kernel_spmd
from concourse.masks import make_identity

N_CORES = 8
N_TRUE = 100000
P = 1024
ROWS_PER_CORE = 12544      # 98 tiles of 128
NT = ROWS_PER_CORE // 128  # 98
K_OUT = 8
SCALE = 1.0 / float(N_TRUE)

F32 = mybir.dt.float32
BF16 = mybir.dt.bfloat16
Alu = mybir.AluOpType
Act = mybir.ActivationFunctionType


def _build(rows_per_core=ROWS_PER_CORE, stage=5):
    nt_total = rows_per_core // 128
    nc = bacc.Bacc("TRN2", target_bir_lowering=False, debug=False,
                   num_devices=N_CORES)
    X = nc.dram_tensor("X", [rows_per_core, P], F32, kind="ExternalInput")
    MU = nc.dram_tensor("mu", [1, 5 * P], F32, kind="ExternalInput")
    W = nc.dram_tensor("W", [5 * P, K_OUT], F32, kind="ExternalInput")
    OUT = nc.dram_tensor("out", [1, K_OUT], F32, kind="ExternalOutput")

    cc_in = nc.dram_tensor("cc_in", [5 * P], F32)
    cc_out = nc.dram_tensor("cc_out", [N_CORES * 5 * P], F32,
                            addr_space="Shared")
    warm_in = nc.dram_tensor("warm_in", [8], F32)
    warm_out = nc.dram_tensor("warm_out", [8], F32, addr_space="Shared")

    # DMA blocks of 4 tiles; compute sub-blocks of 4 tiles
    dbs = [(i, min(4, nt_total - i)) for i in range(0, nt_total, 4)]
    NDIAG = 5                 # S4 via PE diag for cols < 640
    X4LO = NDIAG * 128        # S4 via ACT x^4 + plain reduce for cols >= 640
    X4W = P - X4LO            # 384

    with tile.TileContext(nc) as tc:
        with (
            tc.tile_pool(name="xin", bufs=6) as xin,
            tc.tile_pool(name="pows", bufs=3) as pows,
            tc.tile_pool(name="const", bufs=1) as const,
            tc.tile_pool(name="acc", bufs=1, space="PSUM") as accp,
            tc.tile_pool(name="tailps", bufs=1, space="PSUM") as tailps,
            tc.tile_pool(name="tail", bufs=1) as tailp,
        ):
            ones = const.tile([128, 1], BF16)
            nc.vector.memset(ones, 1.0)
            ident = const.tile([128, 128], F32)
            make_identity(nc, ident)
            # identity replicated along free axis for one-shot S4 extract
            ident_rep = const.tile([128, NDIAG, 128], F32)
            for c in range(NDIAG):
                nc.vector.tensor_copy(ident_rep[:, c, :], ident)

            # tiny dummy collective launched immediately: absorbs the
            # ncfw/TOPSP cold-start barrier (~50-80us) under the main loop
            # so the real AllGather at the tail runs warm (~7us).
            wtile = const.tile([1, 8], F32)
            nc.vector.memset(wtile, 0.0)
            nc.sync.dma_start(out=warm_in.ap(), in_=wtile[0:1, :])
            nc.gpsimd.collective_compute(
                "AllReduce", Alu.add,
                replica_groups=[list(range(N_CORES))],
                ins=[warm_in.ap()],
                outs=[warm_out.ap()],
            )

            # weights / mu for the tail (contraction row j5 = 40*p + q)
            w_s = const.tile([128, 40, K_OUT], F32)
            nc.sync.dma_start(out=w_s, in_=W.ap().rearrange(
                "(p q) k -> p q k", p=128))
            mu_s = const.tile([128, 40], F32)
            nc.sync.dma_start(out=mu_s, in_=MU.ap()[0, :].rearrange(
                "(p q) -> p q", p=128))

            # PSUM accumulators, alive across the whole main loop
            ps_plain = accp.tile([128, P], F32)   # S1@p0 S2@p32 S3@p64 S5@p96
            ps_diag = accp.tile([128, X4LO], F32)  # S4 diag blocks, cols<640
            ps_x4 = accp.tile([1, X4W], F32)       # S4 plain reduce, cols>=640

            Xv = X.ap()

            for t0, ndt in dbs:
                x = xin.tile([128, ndt, P], BF16, tag="x")
                # partition p holds ndt CONSECUTIVE rows -> 4KB*ndt contiguous
                # DMA runs per partition (row->partition placement is free
                # for column sums)
                nc.gpsimd.dma_start(
                    out=x,
                    in_=Xv[t0 * 128:(t0 + ndt) * 128, :].rearrange(
                        "(p s) c -> p s c", s=ndt),
                )
                for half in range(0, ndt, 4):
                    nt = min(4, ndt - half)
                    xh = x[:, half:half + nt, :]
                    sq = pows.tile([128, nt, P], BF16, tag="sq")
                    nc.scalar.activation(sq, xh, Act.Square)
                    cu = pows.tile([128, nt, P], BF16, tag="cu")
                    nc.vector.tensor_mul(cu, xh, sq)
                    x5 = pows.tile([128, nt, P], BF16, tag="x5")
                    nc.vector.tensor_mul(x5, sq, cu)
                    x4 = pows.tile([128, nt, X4W], BF16, tag="x4")
                    nc.scalar.activation(x4, sq[:, :, X4LO:], Act.Square)

                    for t in range(nt):
                        gt = t0 + half + t
                        start = gt == 0
                        stop = gt == nt_total - 1
                        # plain col-sums: 4 powers on 4 concurrent col-strips
                        for h in range(2):
                            sl = slice(h * 512, (h + 1) * 512)
                            for j, pw in enumerate((xh, sq, cu, x5)):
                                bp = 32 * j
                                nc.tensor.matmul(
                                    ps_plain[bp:bp + 1, sl], ones[:, 0:1],
                                    pw[:, t, sl],
                                    start=start, stop=stop,
                                    tile_position=(0, bp),
                                )
                        # S4 tail columns: plain reduce of x^4 (N=384)
                        nc.tensor.matmul(
                            ps_x4[0:1, :], ones[:, 0:1], x4[:, t, :],
                            start=start, stop=stop, tile_position=(0, 0),
                        )
                        # S4 head columns: diag blocks x2_chunk^T @ x2_chunk
                        # start/stop only on first/last matmul per PSUM bank
                        # (chunks 0-3 -> bank 0, chunk 4 -> bank 1)
                        for c in range(NDIAG):
                            cs = slice(c * 128, (c + 1) * 128)
                            nc.tensor.matmul(
                                ps_diag[:, cs], sq[:, t, cs], sq[:, t, cs],
                                start=start and c % 4 == 0,
                                stop=stop and (c == 3 or c == 4),
                                tile_position=(0, 0),
                            )

            # ---- tail ----
            # PSUM->SBUF scaled row copies, split across DVE and ACT so they
            # run in parallel (ACT is otherwise idle in the tail)
            srows = tailp.tile([128, P], F32)
            for jj in (0, 1):
                r = slice(32 * jj, 32 * jj + 1)
                nc.vector.tensor_scalar_mul(srows[r, :], ps_plain[r, :], SCALE)
            for jj in (2, 3):
                r = slice(32 * jj, 32 * jj + 1)
                nc.scalar.activation(srows[r, :], ps_plain[r, :], Act.Copy,
                                     scale=SCALE)

            if stage >= 2:
                s4_s = tailp.tile([128, NDIAG], F32)
                dummy = tailp.tile([128, NDIAG, 128], F32)
                nc.vector.scalar_tensor_tensor(
                    dummy, ps_diag[:].rearrange("p (c i) -> p c i", i=128),
                    SCALE, ident_rep, Alu.mult, Alu.mult)
                nc.vector.tensor_reduce(
                    s4_s, dummy, axis=mybir.AxisListType.X, op=Alu.add)
                s4row = tailp.tile([1, X4W], F32)
                nc.scalar.activation(s4row, ps_x4, Act.Copy, scale=SCALE)

            if stage >= 3:
                # stage scaled raw moments to DRAM: [M1|M2|M3|M4|M5] by column
                for jj, k in ((0, 0), (1, 1), (2, 2), (3, 4)):
                    nc.gpsimd.dma_start(
                        out=cc_in.ap()[k * P:(k + 1) * P],
                        in_=srows[32 * jj:32 * jj + 1, :],
                    )
                nc.gpsimd.dma_start(
                    out=cc_in.ap()[3 * P:3 * P + X4LO].rearrange(
                        "(c i) -> i c", i=128),
                    in_=s4_s,
                )
                nc.gpsimd.dma_start(
                    out=cc_in.ap()[3 * P + X4LO:4 * P],
                    in_=s4row[0:1, :],
                )

                nc.gpsimd.collective_compute(
                    "AllGather", Alu.bypass,
                    replica_groups=[list(range(N_CORES))],
                    ins=[cc_in.ap()],
                    outs=[cc_out.ap()],
                )

                # gathered per-core moments -> sum over cores on DVE
                # momg[p, k, core, cc] = cc_out[core*5120 + k*1024 + 8p + cc]
                momg = tailp.tile([128, 5, N_CORES, K_OUT], F32)
                ccv = cc_out.ap().rearrange(
                    "(r k p c) -> p k r c", r=N_CORES, k=5, p=128)
                for k in range(5):
                    nc.sync.dma_start(out=momg[:, k, :, :], in_=ccv[:, k, :, :])
                nc.vector.tensor_add(momg[:, :, 0:4, :], momg[:, :, 0:4, :],
                                     momg[:, :, 4:8, :])
                nc.vector.tensor_add(momg[:, :, 0:2, :], momg[:, :, 0:2, :],
                                     momg[:, :, 2:4, :])
                nc.vector.tensor_add(momg[:, :, 0:1, :], momg[:, :, 0:1, :],
                                     momg[:, :, 1:2, :])
                # global moments view, (128, 5, 8): [p, k, cc]
                mom = momg[:, :, 0, :]

            if stage >= 4:
                m = mom[:, 0, :]
                M2 = mom[:, 1, :]
                M3 = mom[:, 2, :]
                M4 = mom[:, 3, :]
                M5 = mom[:, 4, :]

                stt = nc.vector.scalar_tensor_tensor
                scr = tailp.tile([128, 12, 8], F32)  # scratch (128,8) slots
                m2, m3, m5, a2, a3, a4, mu2, mu3, b1, c3, c4, t1 = (
                    scr[:, i, :] for i in range(12))

                nc.vector.tensor_mul(m2, m, m)                   # m^2
                nc.vector.tensor_mul(m3, m2, m)                  # m^3
                nc.vector.tensor_mul(m5, m2, m3)                 # m^5
                nc.vector.tensor_sub(mu2, M2, m2)                # mu2 = M2-m^2
                # mu3 = M3 + (-3 M2)*m + 2 m^3
                stt(b1, M2, -3.0, m, Alu.mult, Alu.mult)         # -3 m M2
                nc.vector.tensor_add(b1, b1, M3)
                stt(mu3, m3, 2.0, b1, Alu.mult, Alu.add)         # +2m^3
                # c3 = mu3 - 3 mu2^2
                stt(c3, mu2, -3.0, mu2, Alu.mult, Alu.mult)
                nc.vector.tensor_add(c3, c3, mu3)
                # mu5 = M5 - 5 m M4 + 10 m^2 M3 - 10 m^3 M2 + 4 m^5
                stt(a4, M4, -5.0, m, Alu.mult, Alu.mult)
                stt(a3, M3, 10.0, m2, Alu.mult, Alu.mult)
                stt(a2, M2, -10.0, m3, Alu.mult, Alu.mult)
                nc.vector.tensor_add(a4, a4, M5)
                stt(a3, m5, 4.0, a3, Alu.mult, Alu.add)
                nc.vector.tensor_add(a4, a4, a3)
                nc.vector.tensor_add(a4, a4, a2)                 # mu5
                # c4 = mu5 - 10 mu2 mu3
                stt(t1, mu2, -10.0, mu3, Alu.mult, Alu.mult)
                nc.vector.tensor_add(c4, a4, t1)

                # assemble v[p, q] = X_cum[5*(8p+cc)+k] - mu ; q = 5*cc + k
                v = tailp.tile([128, 40], F32)
                nc.vector.memset(v, 0.0)
                vv = v[:].rearrange("p (c k) -> p c k", k=5)
                nc.vector.tensor_copy(vv[:, :, 0], m)
                nc.vector.tensor_copy(vv[:, :, 2], mu2)
                nc.vector.tensor_copy(vv[:, :, 3], c3)
                nc.vector.tensor_copy(vv[:, :, 4], c4)
                nc.vector.tensor_sub(v, v, mu_s)

            if stage >= 5:
                ps_out = tailps.tile([1, K_OUT], F32)
                for q in range(40):
                    nc.tensor.matmul(
                        ps_out[0:1, :], v[:, q:q + 1], w_s[:, q, :],
                        start=(q == 0), stop=(q == 39),
                    )
                o_s = tailp.tile([1, K_OUT], F32)
                nc.vector.tensor_copy(o_s, ps_out)
                nc.sync.dma_start(out=OUT.ap(), in_=o_s)
            else:
                nc.sync.dma_start(out=OUT.ap(), in_=srows[0:1, 0:K_OUT])

    nc.compile()
    return nc


_NC = None


def _get_nc():
    global _NC
    if _NC is None:
        _NC = _build()
    return _NC


def _shard(X, mu, W):
    Xp = np.zeros((N_CORES * ROWS_PER_CORE, P), dtype=np.float32)
    Xp[:X.shape[0]] = X
    return [
        {
            "X": np.ascontiguousarray(Xp[i * ROWS_PER_CORE:(i + 1) * ROWS_PER_CORE]),
            "mu": np.ascontiguousarray(mu.astype(np.float32)),
            "W": np.ascontiguousarray(W.astype(np.float32)),
        }
        for i in range(N_CORES)
    ]


def run(X, mu, W, trace=False, **trace_kwargs):
    nc = _get_nc()
    in_maps = _shard(np.asarray(X, dtype=np.float32), np.asarray(mu),
                     np.asarray(W))
    res = run_bass_kernel_spmd(nc, in_maps, core_ids=list(range(N_CORES)),
                               trace=trace, **trace_kwargs)
    return res


def kernel(X, mu, W):
    res = run(X, mu, W, trace=False)
    return np.asarray(res.results[0]["out"], dtype=np.float32)
